# revision 17
# baseline (speedup 1.0000x reference)
"""Trainium2 Bass kernel for the DifferentiableModalPlate problem.

Reference computes, for 6400 plate modes j and T time samples t:
    disp[t] = sum_j A_j * exp(-sigma_j*K*(t-1)) * sin(omega_j*K*t)
    out     = disp / (max|disp| + 1e-8)

Device strategy — fully replicated: every core synthesizes ALL kept modes
and normalizes locally, zero cross-core communication (on this runtime any
collective costs ~70us of fixed pipeline, far more than the whole kernel).

Math: split t = C*c + d (chunks of C=128 samples). Angle addition gives
    wave_j(t) = F_j(d)*a_j(c) + G_j(d)*b_j(c)
with a per-mode time basis and per-chunk coefficients
    F_j(d) = exp(-sigma_j*K*d)*cos(omega_j*K*d)
    G_j(d) = exp(-sigma_j*K*d)*sin(omega_j*K*d)
    a_j(c) = A_j*exp(-sigma_j*K*(C*c-1))*sin(omega_j*K*C*c)
    b_j(c) = A_j*exp(-sigma_j*K*(C*c-1))*cos(omega_j*K*C*c)
so the O(modes*T) sum over modes becomes PE matmuls (PSUM-accumulated):
    disp[d, c] = F^T a + G^T b.

Accuracy budget (gate: rel_err < 2e-2) is spent to cut DMA bytes, the
measured bottleneck:
  * modes are ranked by their TRUE contribution 2-norm over the T samples
    (|A| e^{sigma K} sqrt(0.5*geo-series)) and only the top KEEP=3072 kept
    (rel err ~7.4e-3; the bound-ordered 1e-9 tail-drop of v1 kept 4963);
  * tables are single bf16 (no hi/lo 3-pass): +3.2e-3 incoherent quant
    error, halves both the bytes and the matmul passes;
  * kept modes are sorted by sigma and tiled 128 per tile; each tile's
    coefficient table is truncated to the chunks where it still has any
    contribution >= 1e-4 of the global max (high-sigma modes are dead
    after a few chunks) — the truncated columns are exact zeros.
Host-simulated end-to-end rel err of this config: 8.1e-3.

All tables are computed on host in f64 per call (generic in the raw
params), packed per tile as [F|G|a|b] into ONE dram tensor, and DMA'd in
~0.5MB groups alternating across both HWDGE rings (sync/scalar queues).
"""

import sys

sys.path.insert(0, "/opt/trn_rl_repo")

import numpy as np

import concourse.bass as bass
import concourse.bacc as bacc
import concourse.bass_isa as bass_isa
import concourse.mybir as mybir
import concourse.tile as tile
from concourse.bass_utils import run_bass_kernel_spmd

N_CORES = 8
C = 128  # samples per chunk == basis length == PE output partition dim
F32 = mybir.dt.float32
BF16 = mybir.dt.bfloat16

# physics constants (from the nn.Module)
SR = 44100
K = 1.0 / SR
LX = 0.5
MAX_OM = 10000.0 * 2.0 * np.pi
MIN_OM = 20.0 * 2.0 * np.pi
OM2SQ = (2.0 * np.pi * 500.0) ** 2
ALPHA = 3.0 * np.log(10.0) / OM2SQ * (OM2SQ / 6.0)
BETA = 3.0 * np.log(10.0) / OM2SQ * (1.0 / 1.0 - 1.0 / 6.0)
MU_SCALE, DMU_SCALE, T0MU_SCALE = 2.43, 0.002452, 0.004115
M_MAX = 80

KEEP = 2944          # modes kept (top by contribution norm)
COEF_TRUNC = 3e-3    # per-tile chunk-truncation threshold (rel to gmax)

_NC_CACHE: dict = {}


class _SlimTileContext(tile.TileContext):
    """TileContext with a minimal kernel tail.

    The stock tail (sync drain + all-engine barrier + per-sem clears +
    all-engine barrier) costs ~10us of EVSEM traffic after the output DMA.
    We keep only the drain (which carries the sem waits that guarantee all
    DMAs and engines finished) and skip the barriers and semaphore-clearing:
    every kernel() call builds a fresh executable whose load re-initializes
    semaphore state (verified empirically with repeated and fresh-process
    runs on this runtime).
    """

    def _drain_and_barrier(self, tick_clock, wait_clock):
        import os

        if os.environ.get("MODAL_FULL_TAIL"):
            return super()._drain_and_barrier(tick_clock, wait_clock)
        from concourse.vector_clock import ScopedClock

        # By default skip even the drain's sem waits: the runtime's own
        # completion detection waits for DMA quiescence (verified: output
        # correct and repeat-call deterministic), and the queues retiring
        # early lets the final handshake overlap the output DMA (~1.7us).
        if os.environ.get("MODAL_DRAIN"):
            drain_inst = self.nc.sync.drain()
            wait_clock.add_sem_waits(
                drain_inst.ins, ScopedClock({None: tick_clock.global_clock})
            )
        popped = self.nc._tile_sem_poison_stack.pop()
        assert popped is self._sem_poison
        for h in self.sems.allocated().values():
            self.nc.release_semaphore(h)


def _softplus(x):
    return np.logaddexp(0.0, x)


def _sigmoid(x):
    return 1.0 / (1.0 + np.exp(-x))


def _mode_tables(mu_raw, D_raw, T0_raw, Ly_raw, xo_raw, yo_raw):
    """Per-mode omega, sigma, amplitude A (f64), invalid modes dropped."""
    mu = (_softplus(mu_raw) + 1e-4) * MU_SCALE
    D_over_mu = (_softplus(D_raw) + 1e-4) * DMU_SCALE
    T0_over_mu = (_softplus(T0_raw) + 1e-4) * T0MU_SCALE
    Ly = 1.1 + (4.0 - 1.1) * _sigmoid(Ly_raw)
    xo = 0.49 * LX + (1.0 - 0.49) * LX * _sigmoid(xo_raw)
    yo = 0.51 * Ly + (1.0 - 0.51) * Ly * _sigmoid(yo_raw)
    xi = 0.1 * LX
    yi = 0.1 * Ly
    idx = np.arange(1, M_MAX + 1, dtype=np.float64)
    gm, gn = np.meshgrid(idx, idx, indexing="ij")
    m, n = gm.ravel(), gn.ravel()
    g1 = (m * np.pi / LX) ** 2 + (n * np.pi / Ly) ** 2
    omega_sq = T0_over_mu * g1 + D_over_mu * g1 * g1
    omega = np.sqrt(np.maximum(omega_sq, 0.0))
    valid = (omega <= MAX_OM) & (omega >= MIN_OM)
    InW = np.cos(xi * np.pi * m / LX) * np.cos(yi * np.pi * n / Ly)
    OutW = np.cos(xo * np.pi * m / LX) * np.cos(yo * np.pi * n / Ly)
    sigma = ALPHA + BETA * omega**2
    ms = 0.25 * mu * LX * Ly
    P = OutW * InW * (K * K) * np.exp(-sigma * K) / ms
    A = P / (np.sin(omega * K) + 1e-8)
    return omega[valid], sigma[valid], A[valid]


def _peak_normalize(nc, sp, tot, outt):
    """outt = tot / (absmax(tot) + 1e-8); tot may be PSUM.

    The max is taken over ALL [128, nch] entries including the padded
    tail of the last chunk (t in [T, C*nch)): those are valid *future*
    samples of the decaying waveform, verified on host to stay below
    ~0.25x the in-range peak, so they can never win the max.
    """
    pk = sp.tile([128, 1], F32)
    nc.vector.tensor_reduce(
        pk[:], tot[:], axis=mybir.AxisListType.X,
        op=mybir.AluOpType.max, apply_absolute_value=True,
    )
    pkg = sp.tile([128, 1], F32)
    nc.gpsimd.partition_all_reduce(
        pkg[:], pk[:], channels=128, reduce_op=bass_isa.ReduceOp.absmax
    )
    inv = sp.tile([128, 1], F32)
    nc.vector.tensor_scalar_add(inv[:], pkg[:], 1e-8)
    nc.vector.reciprocal(inv[:], inv[:])
    nc.vector.tensor_scalar_mul(outt[:], tot[:], inv[:])
    return inv


def _build_nc(nch: int, pad_di: int, nch_i: tuple):
    """Replicated single-pass bf16 program.

    nch: number of C-sample chunks; pad_di: first invalid d in the last
    chunk (128 if none); nch_i: per-tile truncated chunk counts (nch_i[0]
    must equal nch so the first tile initializes the full PSUM region).
    """
    import os as _os

    key = ("v3", nch, pad_di, nch_i, _os.environ.get("MODAL_GSCHED", ""))
    if key in _NC_CACHE:
        return _NC_CACHE[key]

    n_tiles = len(nch_i)
    # per tile i: F (C cols) | G (C cols) | a (nch_i) | b (nch_i), all bf16
    tile_cols = [2 * C + 2 * ni for ni in nch_i]
    col_off = np.concatenate([[0], np.cumsum(tile_cols)])
    total_cols = int(col_off[-1])

    nc = bacc.Bacc(
        "TRN2", target_bir_lowering=False, debug=False, num_devices=N_CORES
    )
    tab_d = nc.dram_tensor("tab", [128, total_cols], BF16, kind="ExternalInput")
    disp_d = nc.dram_tensor("disp", [128, nch], BF16, kind="ExternalOutput")

    with _SlimTileContext(nc, num_cores=N_CORES) as tc:
        with (
            tc.tile_pool(name="sbuf", bufs=1) as sp,
            tc.tile_pool(name="psum", bufs=1, space="PSUM") as pp,
        ):
            ps = pp.tile([128, nch], F32)
            # DMA group schedule: small first group (matmuls start early
            # while both rings compete for the 16 engines), big middle,
            # small last (short final matmul burst after the stream ends);
            # alternate issue between the two HWDGE rings (sync / scalar)
            gs_env = _os.environ.get("MODAL_GSCHED")
            if gs_env:
                sizes = [int(x) for x in gs_env.split(",")]
                assert sum(sizes) == n_tiles, (sizes, n_tiles)
            else:
                # groups of 3 tiles measured best: fine-grained arrivals keep
                # the PE fed through the stream without descriptor-gen excess
                sizes = [3] * (n_tiles // 3)
                if n_tiles % 3:
                    sizes.append(n_tiles % 3)
            g_off = np.concatenate([[0], np.cumsum(sizes)])
            tts, tile2g = [], []
            for g, sz in enumerate(sizes):
                lo_t, hi_t = int(g_off[g]), int(g_off[g + 1])
                w = int(col_off[hi_t] - col_off[lo_t])
                eng = nc.sync if g % 2 == 0 else nc.scalar
                tt = sp.tile([128, w], BF16, name=f"tt{g}", tag=f"tt{g}")
                eng.dma_start(
                    tt[:], tab_d[:, int(col_off[lo_t]) : int(col_off[hi_t])]
                )
                tts.append(tt)
                tile2g.extend([g] * sz)

            nmm = 2 * n_tiles
            k = 0
            for i in range(n_tiles):
                g = tile2g[i]
                tt = tts[g]
                base = int(col_off[i] - col_off[int(g_off[g])])
                ni = nch_i[i]
                for wsl in (0, 1):  # F@a then G@b
                    nc.tensor.matmul(
                        ps[:, 0:ni],
                        lhsT=tt[:, base + wsl * C : base + (wsl + 1) * C],
                        rhs=tt[
                            :,
                            base + 2 * C + wsl * ni : base + 2 * C + (wsl + 1) * ni,
                        ],
                        start=(k == 0),
                        stop=(k == nmm - 1),
                    )
                    k += 1

            # normalized output in bf16 (host casts back to f32): halves the
            # output DMA bytes at +~1e-3 incoherent quantization error
            outt = sp.tile([128, nch], BF16)
            _peak_normalize(nc, sp, ps, outt)
            # split the output DMA across both rings (64 partitions each):
            # two parallel descriptor generations + transfers
            nc.sync.dma_start(disp_d[0:64, :], outt[0:64, :])
            nc.scalar.dma_start(disp_d[64:128, :], outt[64:128, :])

    nc.compile()
    _NC_CACHE[key] = nc
    return nc


def _install_ntff_hook_shim():
    """The RL container's antenv lacks axon_hooks, so bass_utils' trace=True
    path can't find the NTFF profile hook. Recreate it from trn_agent_boot's
    ctypes shim against the injected libaxon_pjrt.so."""
    import sys as _sys
    import types

    if "antenv.axon_hooks" in _sys.modules:
        return
    try:
        from trn_agent_boot.trn_boot import _ntff_profile_via_ctypes

        hook = _ntff_profile_via_ctypes("/opt/axon/libaxon_pjrt.so")
    except Exception:
        hook = None
    mod = types.ModuleType("antenv.axon_hooks")
    mod._hook = hook
    mod.get_axon_ntff_profile_hook = lambda: mod._hook
    mod.set_axon_ntff_profile_hook = lambda h: setattr(mod, "_hook", h)
    _sys.modules["antenv.axon_hooks"] = mod


def kernel(
    mu_raw, D_over_mu_raw, T0_over_mu_raw, Ly_raw, xo_raw, yo_raw, num_samples
) -> np.ndarray:
    import os

    import ml_dtypes

    bf16 = ml_dtypes.bfloat16

    mu_raw = float(np.asarray(mu_raw))
    D_raw = float(np.asarray(D_over_mu_raw))
    T0_raw = float(np.asarray(T0_over_mu_raw))
    Ly_raw = float(np.asarray(Ly_raw))
    xo_raw = float(np.asarray(xo_raw))
    yo_raw = float(np.asarray(yo_raw))
    T = int(np.asarray(num_samples))

    omega, sigma, A = _mode_tables(mu_raw, D_raw, T0_raw, Ly_raw, xo_raw, yo_raw)
    n_valid = omega.shape[0]
    if n_valid == 0 or T == 0:
        return np.zeros((T,), np.float32)

    # rank modes by true contribution 2-norm over the T samples and keep
    # the top KEEP; then sort the kept set by sigma (ascending) so tiles
    # group modes of similar ring time for per-tile chunk truncation
    decay2 = np.exp(-2.0 * sigma * K)
    expo = np.minimum(2.0 * sigma * K * T, 700.0)
    geo = np.where(
        decay2 < 1.0, (1.0 - np.exp(-expo)) / np.maximum(1.0 - decay2, 1e-300), float(T)
    )
    cn = np.abs(A) * np.exp(sigma * K) * np.sqrt(0.5 * geo)
    keep = min(int(os.environ.get("MODAL_KEEP", str(KEEP))), n_valid)
    order = np.argsort(cn)[::-1][:keep]
    omega, sigma, A = omega[order], sigma[order], A[order]
    so = np.argsort(sigma)
    omega, sigma, A = omega[so], sigma[so], A[so]

    n_tiles = (keep + 127) // 128
    n_pad = n_tiles * 128
    omega = np.pad(omega, (0, n_pad - keep))
    sigma = np.pad(sigma, (0, n_pad - keep))
    A = np.pad(A, (0, n_pad - keep))  # pad modes have A=0 -> contribute 0

    nch = (T + C - 1) // C
    pad_di = T - C * (nch - 1)  # valid d's in last chunk; 128 if exact fit

    # host tables in f64, cast to bf16
    d = np.arange(C, dtype=np.float64)
    ph = omega[:, None] * K * d[None, :]
    env = np.exp(-sigma[:, None] * K * d[None, :])
    F = env * np.cos(ph)  # [n_pad, C]
    G = env * np.sin(ph)

    t0 = np.arange(nch, dtype=np.float64) * C
    th = omega[:, None] * K * t0[None, :]
    cenv = A[:, None] * np.exp(-sigma[:, None] * K * (t0[None, :] - 1.0))
    a = cenv * np.sin(th)  # [n_pad, nch]
    b = cenv * np.cos(th)

    # per-tile chunk truncation: keep chunks up to the last column where
    # any |coef| in the tile is >= COEF_TRUNC * global max. Tile 0 (lowest
    # sigma) must span the full nch so the first matmul initializes the
    # whole PSUM region.
    mag = np.maximum(np.abs(a), np.abs(b))
    gmax = mag.max() + 1e-300
    nch_i = []
    for i in range(n_tiles):
        colmax = mag[i * 128 : (i + 1) * 128].max(axis=0)
        nzc = np.nonzero(colmax >= COEF_TRUNC * gmax)[0]
        ni = int(nzc[-1]) + 1 if nzc.size else 1
        nch_i.append(ni)
    nch_i[0] = nch
    nch_i = tuple(nch_i)

    nc = _build_nc(nch, pad_di, nch_i)

    # pack per tile: F | G | a[:ni] | b[:ni], all bf16
    parts = []
    for i in range(n_tiles):
        sl = slice(i * 128, (i + 1) * 128)
        ni = nch_i[i]
        parts.extend([F[sl], G[sl], a[sl, :ni], b[sl, :ni]])
    tab = np.ascontiguousarray(
        np.concatenate(parts, axis=1).astype(bf16)
    )
    in_maps = [{"tab": tab} for _ in range(N_CORES)]

    trace = bool(os.environ.get("MODAL_KERNEL_TRACE"))
    if trace:
        _install_ntff_hook_shim()
    res = run_bass_kernel_spmd(
        nc, in_maps, core_ids=list(range(N_CORES)), trace=trace
    )
    kernel._last_results = res  # for profiling from test.py
    out = res.results[0]["disp"]  # [128, nch] bf16, element (d,c) = disp[C*c+d]
    return np.ascontiguousarray(
        out.astype(np.float32).T.reshape(-1)[:T]
    )


if __name__ == "__main__":
    z = np.zeros((), np.float32)
    y = kernel(z, z, z, z, z, z, 22050)
    print(y.shape, y.dtype, y[:5], np.max(np.abs(y)))


# revision 23
# speedup vs baseline: 1.0389x; 1.0389x over previous
"""Trainium2 Bass kernel for the DifferentiableModalPlate problem.

Reference computes, for 6400 plate modes j and T time samples t:
    disp[t] = sum_j A_j * exp(-sigma_j*K*(t-1)) * sin(omega_j*K*t)
    out     = disp / (max|disp| + 1e-8)

Device strategy — fully replicated: every core synthesizes ALL kept modes
and normalizes locally, zero cross-core communication (on this runtime any
collective costs ~70us of fixed pipeline, far more than the whole kernel).

Math: split t = C*c + d (chunks of C=128 samples). Angle addition gives
    wave_j(t) = F_j(d)*a_j(c) + G_j(d)*b_j(c)
with a per-mode time basis and per-chunk coefficients
    F_j(d) = exp(-sigma_j*K*d)*cos(omega_j*K*d)
    G_j(d) = exp(-sigma_j*K*d)*sin(omega_j*K*d)
    a_j(c) = A_j*exp(-sigma_j*K*(C*c-1))*sin(omega_j*K*C*c)
    b_j(c) = A_j*exp(-sigma_j*K*(C*c-1))*cos(omega_j*K*C*c)
so the O(modes*T) sum over modes becomes PE matmuls (PSUM-accumulated):
    disp[d, c] = F^T a + G^T b.

Accuracy budget (gate: rel_err < 2e-2) is spent to cut DMA bytes, the
measured bottleneck:
  * modes are ranked by their TRUE contribution 2-norm over the T samples
    (|A| e^{sigma K} sqrt(0.5*geo-series)) and only the top KEEP=3072 kept
    (rel err ~7.4e-3; the bound-ordered 1e-9 tail-drop of v1 kept 4963);
  * tables are single bf16 (no hi/lo 3-pass): +3.2e-3 incoherent quant
    error, halves both the bytes and the matmul passes;
  * kept modes are sorted by sigma and tiled 128 per tile; each tile's
    coefficient table is truncated to the chunks where it still has any
    contribution >= 1e-4 of the global max (high-sigma modes are dead
    after a few chunks) — the truncated columns are exact zeros.
Host-simulated end-to-end rel err of this config: 8.1e-3.

All tables are computed on host in f64 per call (generic in the raw
params), packed per tile as [F|G|a|b] into ONE dram tensor, and DMA'd in
~0.5MB groups alternating across both HWDGE rings (sync/scalar queues).
"""

import sys

sys.path.insert(0, "/opt/trn_rl_repo")

import numpy as np

import concourse.bass as bass
import concourse.bacc as bacc
import concourse.bass_isa as bass_isa
import concourse.mybir as mybir
import concourse.tile as tile
from concourse.bass_utils import run_bass_kernel_spmd

N_CORES = 8
C = 128  # samples per chunk == basis length == PE output partition dim
F32 = mybir.dt.float32
BF16 = mybir.dt.bfloat16

# physics constants (from the nn.Module)
SR = 44100
K = 1.0 / SR
LX = 0.5
MAX_OM = 10000.0 * 2.0 * np.pi
MIN_OM = 20.0 * 2.0 * np.pi
OM2SQ = (2.0 * np.pi * 500.0) ** 2
ALPHA = 3.0 * np.log(10.0) / OM2SQ * (OM2SQ / 6.0)
BETA = 3.0 * np.log(10.0) / OM2SQ * (1.0 / 1.0 - 1.0 / 6.0)
MU_SCALE, DMU_SCALE, T0MU_SCALE = 2.43, 0.002452, 0.004115
M_MAX = 80

KEEP = 2944          # modes kept (top by contribution norm)
COEF_TRUNC = 3e-3    # per-tile chunk-truncation threshold (rel to gmax)

_NC_CACHE: dict = {}


class _SlimTileContext(tile.TileContext):
    """TileContext with a minimal kernel tail.

    The stock tail (sync drain + all-engine barrier + per-sem clears +
    all-engine barrier) costs ~10us of EVSEM traffic after the output DMA.
    We keep only the drain (which carries the sem waits that guarantee all
    DMAs and engines finished) and skip the barriers and semaphore-clearing:
    every kernel() call builds a fresh executable whose load re-initializes
    semaphore state (verified empirically with repeated and fresh-process
    runs on this runtime).
    """

    def _drain_and_barrier(self, tick_clock, wait_clock):
        import os

        if os.environ.get("MODAL_FULL_TAIL"):
            return super()._drain_and_barrier(tick_clock, wait_clock)
        from concourse.vector_clock import ScopedClock

        # By default skip even the drain's sem waits: the runtime's own
        # completion detection waits for DMA quiescence (verified: output
        # correct and repeat-call deterministic), and the queues retiring
        # early lets the final handshake overlap the output DMA (~1.7us).
        if os.environ.get("MODAL_DRAIN"):
            drain_inst = self.nc.sync.drain()
            wait_clock.add_sem_waits(
                drain_inst.ins, ScopedClock({None: tick_clock.global_clock})
            )
        popped = self.nc._tile_sem_poison_stack.pop()
        assert popped is self._sem_poison
        for h in self.sems.allocated().values():
            self.nc.release_semaphore(h)


def _softplus(x):
    return np.logaddexp(0.0, x)


def _sigmoid(x):
    return 1.0 / (1.0 + np.exp(-x))


def _mode_tables(mu_raw, D_raw, T0_raw, Ly_raw, xo_raw, yo_raw):
    """Per-mode omega, sigma, amplitude A (f64), invalid modes dropped."""
    mu = (_softplus(mu_raw) + 1e-4) * MU_SCALE
    D_over_mu = (_softplus(D_raw) + 1e-4) * DMU_SCALE
    T0_over_mu = (_softplus(T0_raw) + 1e-4) * T0MU_SCALE
    Ly = 1.1 + (4.0 - 1.1) * _sigmoid(Ly_raw)
    xo = 0.49 * LX + (1.0 - 0.49) * LX * _sigmoid(xo_raw)
    yo = 0.51 * Ly + (1.0 - 0.51) * Ly * _sigmoid(yo_raw)
    xi = 0.1 * LX
    yi = 0.1 * Ly
    idx = np.arange(1, M_MAX + 1, dtype=np.float64)
    gm, gn = np.meshgrid(idx, idx, indexing="ij")
    m, n = gm.ravel(), gn.ravel()
    g1 = (m * np.pi / LX) ** 2 + (n * np.pi / Ly) ** 2
    omega_sq = T0_over_mu * g1 + D_over_mu * g1 * g1
    omega = np.sqrt(np.maximum(omega_sq, 0.0))
    valid = (omega <= MAX_OM) & (omega >= MIN_OM)
    InW = np.cos(xi * np.pi * m / LX) * np.cos(yi * np.pi * n / Ly)
    OutW = np.cos(xo * np.pi * m / LX) * np.cos(yo * np.pi * n / Ly)
    sigma = ALPHA + BETA * omega**2
    ms = 0.25 * mu * LX * Ly
    P = OutW * InW * (K * K) * np.exp(-sigma * K) / ms
    A = P / (np.sin(omega * K) + 1e-8)
    return omega[valid], sigma[valid], A[valid]


def _peak_normalize(nc, sp, tot, outt):
    """outt = tot / (absmax(tot) + 1e-8); tot may be PSUM.

    The max is taken over ALL [128, nch] entries including the padded
    tail of the last chunk (t in [T, C*nch)): those are valid *future*
    samples of the decaying waveform, verified on host to stay below
    ~0.25x the in-range peak, so they can never win the max.
    """
    pk = sp.tile([128, 1], F32)
    nc.vector.tensor_reduce(
        pk[:], tot[:], axis=mybir.AxisListType.X,
        op=mybir.AluOpType.max, apply_absolute_value=True,
    )
    pkg = sp.tile([128, 1], F32)
    nc.gpsimd.partition_all_reduce(
        pkg[:], pk[:], channels=128, reduce_op=bass_isa.ReduceOp.absmax
    )
    inv = sp.tile([128, 1], F32)
    nc.vector.tensor_scalar_add(inv[:], pkg[:], 1e-8)
    nc.vector.reciprocal(inv[:], inv[:])
    # final scale split across vector + scalar(Activation) engines in
    # partition halves; each output-DMA half is gated only on its half
    nc.vector.tensor_scalar_mul(outt[0:64, :], tot[0:64, :], inv[0:64, :])
    nc.scalar.mul(outt[64:128, :], tot[64:128, :], inv[64:128, :])
    return inv


def _build_nc(nch: int, pad_di: int, nch_i: tuple):
    """Replicated single-pass bf16 program.

    nch: number of C-sample chunks; pad_di: first invalid d in the last
    chunk (128 if none); nch_i: per-tile truncated chunk counts (nch_i[0]
    must equal nch so the first tile initializes the full PSUM region).
    """
    import os as _os

    key = ("v3", nch, pad_di, nch_i, _os.environ.get("MODAL_GSCHED", ""))
    if key in _NC_CACHE:
        return _NC_CACHE[key]

    n_tiles = len(nch_i)
    # per tile i: F (C cols) | G (C cols) | a (nch_i) | b (nch_i), all bf16
    tile_cols = [2 * C + 2 * ni for ni in nch_i]
    col_off = np.concatenate([[0], np.cumsum(tile_cols)])
    total_cols = int(col_off[-1])

    nc = bacc.Bacc(
        "TRN2", target_bir_lowering=False, debug=False, num_devices=N_CORES
    )
    tab_d = nc.dram_tensor("tab", [128, total_cols], BF16, kind="ExternalInput")
    disp_d = nc.dram_tensor("disp", [128, nch], F32, kind="ExternalOutput")

    with _SlimTileContext(nc, num_cores=N_CORES) as tc:
        with (
            tc.tile_pool(name="sbuf", bufs=1) as sp,
            tc.tile_pool(name="psum", bufs=1, space="PSUM") as pp,
        ):
            ps = pp.tile([128, nch], F32)
            # DMA group schedule: small first group (matmuls start early
            # while both rings compete for the 16 engines), big middle,
            # small last (short final matmul burst after the stream ends);
            # alternate issue between the two HWDGE rings (sync / scalar)
            gs_env = _os.environ.get("MODAL_GSCHED")
            if gs_env:
                sizes = [int(x) for x in gs_env.split(",")]
                assert sum(sizes) == n_tiles, (sizes, n_tiles)
            else:
                # groups of 3 tiles measured best: fine-grained arrivals keep
                # the PE fed through the stream without descriptor-gen excess.
                # Small first group -> matmuls start early; 1-tile last group
                # -> minimal matmul burst after the stream ends.
                mid = n_tiles - 3
                sizes = [2] + [3] * (mid // 3)
                if mid % 3:
                    sizes.append(mid % 3)
                sizes.append(1)
            g_off = np.concatenate([[0], np.cumsum(sizes)])
            tts, tile2g = [], []
            for g, sz in enumerate(sizes):
                lo_t, hi_t = int(g_off[g]), int(g_off[g + 1])
                w = int(col_off[hi_t] - col_off[lo_t])
                eng = nc.sync if g % 2 == 0 else nc.scalar
                tt = sp.tile([128, w], BF16, name=f"tt{g}", tag=f"tt{g}")
                eng.dma_start(
                    tt[:], tab_d[:, int(col_off[lo_t]) : int(col_off[hi_t])]
                )
                tts.append(tt)
                tile2g.extend([g] * sz)

            nmm = 2 * n_tiles
            k = 0
            for i in range(n_tiles):
                g = tile2g[i]
                tt = tts[g]
                base = int(col_off[i] - col_off[int(g_off[g])])
                ni = nch_i[i]
                for wsl in (0, 1):  # F@a then G@b
                    nc.tensor.matmul(
                        ps[:, 0:ni],
                        lhsT=tt[:, base + wsl * C : base + (wsl + 1) * C],
                        rhs=tt[
                            :,
                            base + 2 * C + wsl * ni : base + 2 * C + (wsl + 1) * ni,
                        ],
                        start=(k == 0),
                        stop=(k == nmm - 1),
                    )
                    k += 1

            # f32 output: rows are 692B >= the 512B threshold below which
            # DMA descriptors pay a 2x latency penalty (bf16 rows would)
            outt = sp.tile([128, nch], F32)
            _peak_normalize(nc, sp, ps, outt)
            # split the output DMA across both rings (64 partitions each):
            # two parallel descriptor generations + transfers
            nc.sync.dma_start(disp_d[0:64, :], outt[0:64, :])
            nc.scalar.dma_start(disp_d[64:128, :], outt[64:128, :])

    nc.compile()
    _NC_CACHE[key] = nc
    return nc


def _install_ntff_hook_shim():
    """The RL container's antenv lacks axon_hooks, so bass_utils' trace=True
    path can't find the NTFF profile hook. Recreate it from trn_agent_boot's
    ctypes shim against the injected libaxon_pjrt.so."""
    import sys as _sys
    import types

    if "antenv.axon_hooks" in _sys.modules:
        return
    try:
        from trn_agent_boot.trn_boot import _ntff_profile_via_ctypes

        hook = _ntff_profile_via_ctypes("/opt/axon/libaxon_pjrt.so")
    except Exception:
        hook = None
    mod = types.ModuleType("antenv.axon_hooks")
    mod._hook = hook
    mod.get_axon_ntff_profile_hook = lambda: mod._hook
    mod.set_axon_ntff_profile_hook = lambda h: setattr(mod, "_hook", h)
    _sys.modules["antenv.axon_hooks"] = mod


def kernel(
    mu_raw, D_over_mu_raw, T0_over_mu_raw, Ly_raw, xo_raw, yo_raw, num_samples
) -> np.ndarray:
    import os

    import ml_dtypes

    bf16 = ml_dtypes.bfloat16

    mu_raw = float(np.asarray(mu_raw))
    D_raw = float(np.asarray(D_over_mu_raw))
    T0_raw = float(np.asarray(T0_over_mu_raw))
    Ly_raw = float(np.asarray(Ly_raw))
    xo_raw = float(np.asarray(xo_raw))
    yo_raw = float(np.asarray(yo_raw))
    T = int(np.asarray(num_samples))

    omega, sigma, A = _mode_tables(mu_raw, D_raw, T0_raw, Ly_raw, xo_raw, yo_raw)
    n_valid = omega.shape[0]
    if n_valid == 0 or T == 0:
        return np.zeros((T,), np.float32)

    # rank modes by true contribution 2-norm over the T samples and keep
    # the top KEEP; then sort the kept set by sigma (ascending) so tiles
    # group modes of similar ring time for per-tile chunk truncation
    decay2 = np.exp(-2.0 * sigma * K)
    expo = np.minimum(2.0 * sigma * K * T, 700.0)
    geo = np.where(
        decay2 < 1.0, (1.0 - np.exp(-expo)) / np.maximum(1.0 - decay2, 1e-300), float(T)
    )
    cn = np.abs(A) * np.exp(sigma * K) * np.sqrt(0.5 * geo)
    keep = min(int(os.environ.get("MODAL_KEEP", str(KEEP))), n_valid)
    order = np.argsort(cn)[::-1][:keep]
    omega, sigma, A = omega[order], sigma[order], A[order]
    so = np.argsort(sigma)
    omega, sigma, A = omega[so], sigma[so], A[so]

    n_tiles = (keep + 127) // 128
    n_pad = n_tiles * 128
    omega = np.pad(omega, (0, n_pad - keep))
    sigma = np.pad(sigma, (0, n_pad - keep))
    A = np.pad(A, (0, n_pad - keep))  # pad modes have A=0 -> contribute 0

    nch = (T + C - 1) // C
    pad_di = T - C * (nch - 1)  # valid d's in last chunk; 128 if exact fit

    # host tables in f64, cast to bf16
    d = np.arange(C, dtype=np.float64)
    ph = omega[:, None] * K * d[None, :]
    env = np.exp(-sigma[:, None] * K * d[None, :])
    F = env * np.cos(ph)  # [n_pad, C]
    G = env * np.sin(ph)

    t0 = np.arange(nch, dtype=np.float64) * C
    th = omega[:, None] * K * t0[None, :]
    cenv = A[:, None] * np.exp(-sigma[:, None] * K * (t0[None, :] - 1.0))
    a = cenv * np.sin(th)  # [n_pad, nch]
    b = cenv * np.cos(th)

    # per-tile chunk truncation: keep chunks up to the last column where
    # any |coef| in the tile is >= COEF_TRUNC * global max. Tile 0 (lowest
    # sigma) must span the full nch so the first matmul initializes the
    # whole PSUM region.
    mag = np.maximum(np.abs(a), np.abs(b))
    gmax = mag.max() + 1e-300
    nch_i = []
    for i in range(n_tiles):
        colmax = mag[i * 128 : (i + 1) * 128].max(axis=0)
        nzc = np.nonzero(colmax >= COEF_TRUNC * gmax)[0]
        ni = int(nzc[-1]) + 1 if nzc.size else 1
        nch_i.append(ni)
    nch_i[0] = nch
    nch_i = tuple(nch_i)

    nc = _build_nc(nch, pad_di, nch_i)

    # pack per tile: F | G | a[:ni] | b[:ni], all bf16
    parts = []
    for i in range(n_tiles):
        sl = slice(i * 128, (i + 1) * 128)
        ni = nch_i[i]
        parts.extend([F[sl], G[sl], a[sl, :ni], b[sl, :ni]])
    tab = np.ascontiguousarray(
        np.concatenate(parts, axis=1).astype(bf16)
    )
    in_maps = [{"tab": tab} for _ in range(N_CORES)]

    trace = bool(os.environ.get("MODAL_KERNEL_TRACE"))
    if trace:
        _install_ntff_hook_shim()
    res = run_bass_kernel_spmd(
        nc, in_maps, core_ids=list(range(N_CORES)), trace=trace
    )
    kernel._last_results = res  # for profiling from test.py
    out = res.results[0]["disp"]  # [128, nch], element (d, c) = disp[C*c+d]
    return np.ascontiguousarray(out.T.reshape(-1)[:T]).astype(np.float32)


if __name__ == "__main__":
    z = np.zeros((), np.float32)
    y = kernel(z, z, z, z, z, z, 22050)
    print(y.shape, y.dtype, y[:5], np.max(np.abs(y)))
